# revision 1
# baseline (speedup 1.0000x reference)
"""3-layer GCN on 8 TRN2 NeuronCores (Bass/Tile).

Strategy (matches the sharding hint): nodes are partitioned across the 8
cores (12500 dst nodes each); each core owns the edges whose dst falls in
its shard. Per layer the core aggregates messages for its dst shard by
gathering source-node feature rows (dma_gather, bf16) and reducing them
into PSUM via one-hot matmuls on the TensorEngine; the dense transforms
(dis*agg @ W + b, relu) run on the shard. Node features for the next
layer are exchanged with an AllGather. GCN normalization is rewritten as

    h_{l+1} = relu( (dis ⊙ (A_w @ (dis ⊙ h_l))) @ W_l + b_l )

(A_w unweighted-except-edge-weight adjacency, dis = deg^-1/2), which is
exact because the dense transform commutes with the aggregation.
Edge weights ride inside the one-hot values (is_equal * w).

The per-(superblock, src-quadrant) chunk schedule is computed jointly
across all 8 cores so a single SPMD program fits every core; cores pad
their chunks (idx=0 rows with zero one-hot weight) where their edge
counts differ.
"""

import os
import sys

sys.path.insert(0, "/opt/trn_rl_repo")

ABLATE = set(os.environ.get("GCN_ABLATE", "").split(","))
_BUFS = {}
for kv in os.environ.get("GCN_BUFS", "").split(","):
    if "=" in kv:
        k, v = kv.split("=")
        _BUFS[k] = int(v)

import numpy as np
import ml_dtypes

import concourse.bacc as bacc
import concourse.mybir as mybir
import concourse.tile as tile
from concourse import library_config
from concourse.bass_utils import run_bass_kernel_spmd

f32 = mybir.dt.float32
bf16 = mybir.dt.bfloat16
i16 = mybir.dt.int16

PB = 128  # partition / block size


def make_cfg(N=100000, D=128, DOUT=64, CORES=8, SB_BLOCKS=8, NQ=4):
    shard = N // CORES
    assert shard * CORES == N
    shard_pad = ((shard + PB - 1) // PB) * PB
    npad = shard_pad * CORES
    assert npad % NQ == 0
    qrows = npad // NQ
    assert qrows <= 32768, "dma_gather int16 index range"
    nblocks = shard_pad // PB
    sb_d = SB_BLOCKS * PB
    nsb = (nblocks + SB_BLOCKS - 1) // SB_BLOCKS
    return dict(
        N=N, D=D, DOUT=DOUT, CORES=CORES, SHARD=shard, SHARD_PAD=shard_pad,
        NPAD=npad, NQ=NQ, QROWS=qrows, NBLOCKS=nblocks, SB_D=sb_d, NSB=nsb,
    )


def pad_row(n, cfg):
    return (n // cfg["SHARD"]) * cfg["SHARD_PAD"] + n % cfg["SHARD"]


def make_schedule(edge_src, edge_dst, edge_w, cfg):
    """Cross-core-uniform chunk schedule.

    Returns (runs, chunks, per-core arrays):
      runs: list of (sb, q, C, idx_off) in emission order
      chunks: list of (sb, q, wbase, flags) where flags = ((start0, stop0),
              (start1, stop1)) for the two 128-wide matmul halves
      idxr [CORES, 128, idxcols] i16, dstv/wval [CORES, 128, Ctot] f32
    """
    CORES, SHARD, SB_D = cfg["CORES"], cfg["SHARD"], cfg["SB_D"]
    QROWS, NSB, NBLOCKS = cfg["QROWS"], cfg["NSB"], cfg["NBLOCKS"]
    NQ = cfg["NQ"]

    per = []  # [c][sb][q] -> (dsl, idxq, wv) sorted by dsl
    for c in range(CORES):
        m = (edge_dst >= c * SHARD) & (edge_dst < (c + 1) * SHARD)
        dl = edge_dst[m] - c * SHARD
        sp = pad_row(edge_src[m], cfg)
        wv = edge_w[m]
        sb = dl // SB_D
        q = sp // QROWS
        order = np.lexsort((dl, q, sb))
        dl, sp, wv, sb, q = dl[order], sp[order], wv[order], sb[order], q[order]
        key = sb * NQ + q
        bounds = np.searchsorted(key, np.arange(NSB * NQ + 1))
        rows = []
        for s in range(NSB):
            qs = []
            for qq in range(NQ):
                lo, hi = bounds[s * NQ + qq], bounds[s * NQ + qq + 1]
                qs.append((
                    (dl[lo:hi] - s * SB_D).astype(np.int32),
                    (sp[lo:hi] - qq * QROWS).astype(np.int32),
                    wv[lo:hi].astype(np.float32),
                ))
            rows.append(qs)
        per.append(rows)

    runs = []
    chunks = []
    ch_dstv, ch_wval = [], []   # per chunk: [CORES,128] arrays
    run_idx_blocks = []         # per run: [CORES, 128, 8*C] i16
    first_mm = {}               # (sb, bank) -> chunk half getting start
    last_mm = {}
    idx_off = 0
    for s in range(NSB):
        ncols = min(SB_D, NBLOCKS * PB - s * SB_D)
        for qq in range(NQ):
            datas = [per[c][s][qq] for c in range(CORES)]
            lens = [d[0].shape[0] for d in datas]
            if max(lens) == 0:
                continue
            ptr = [0] * CORES
            run_chunk_idx = []  # [CORES,128] per chunk
            C = 0
            while True:
                active = [c for c in range(CORES) if ptr[c] < lens[c]]
                if not active:
                    break
                mind = min(int(datas[c][0][ptr[c]]) for c in active)
                wbase = min((mind // PB) * PB, max(0, ncols - 2 * PB))
                dv = np.full((CORES, PB), -1.0, np.float32)
                wv = np.zeros((CORES, PB), np.float32)
                ix = np.zeros((CORES, PB), np.int16)
                for c in range(CORES):
                    dl, iq, ww = datas[c]
                    p = ptr[c]
                    hi = np.searchsorted(dl, wbase + 2 * PB, side="left")
                    take = min(PB, hi - p)
                    if take > 0:
                        dv[c, :take] = dl[p : p + take] - wbase
                        wv[c, :take] = ww[p : p + take]
                        ix[c, :take] = iq[p : p + take]
                        ptr[c] = p + take
                gc = len(chunks)
                used = [
                    bool(((dv >= 0) & (dv < PB)).any()),
                    bool((dv >= PB).any()),
                ]
                flags = []
                for h in range(2):
                    col = wbase + h * PB
                    if col >= ncols or not used[h]:
                        flags.append(None)
                        continue
                    bank = (s, col // 512)
                    st = bank not in first_mm
                    if st:
                        first_mm[bank] = (gc, h)
                    last_mm[bank] = (gc, h)
                    flags.append(st)
                chunks.append([s, qq, wbase, flags])
                ch_dstv.append(dv)
                ch_wval.append(wv)
                run_chunk_idx.append(ix)
                C += 1
                if C > (max(lens) // PB) + NBLOCKS + 16:
                    raise RuntimeError("chunk packing did not converge")
            # idx region for the run: flat [128*C] -> [16, 8C] -> [128, 8C]
            blk = np.zeros((CORES, PB, 8 * C), np.int16)
            for c in range(CORES):
                flat = np.concatenate([ci[c] for ci in run_chunk_idx])
                wrapped = flat.reshape(-1, 16).T  # [16, 8C]
                blk[c] = np.tile(wrapped, (8, 1))
            run_idx_blocks.append(blk)
            runs.append([s, qq, C, idx_off])
            idx_off += 8 * C

    # every (sb, bank) must receive at least one matmul (else stale PSUM)
    for s in range(NSB):
        ncols = min(SB_D, NBLOCKS * PB - s * SB_D)
        for bank in range((ncols + 511) // 512):
            assert (s, bank) in first_mm, f"uncovered psum bank {(s, bank)}"

    # stop flags
    stops = {v: k for k, v in last_mm.items()}
    for gc, ch in enumerate(chunks):
        fl = ch[3]
        ch[3] = tuple(
            None if fl[h] is None else (fl[h], (gc, h) in stops) for h in range(2)
        )

    Ctot = len(chunks)
    dstv = np.stack(ch_dstv, axis=2)  # [CORES, 128, Ctot]
    wval = np.stack(ch_wval, axis=2)
    idxr = np.concatenate(run_idx_blocks, axis=2)  # [CORES, 128, idxcols]
    return runs, chunks, idxr, dstv, wval


def build_program(runs, chunks, cfg, idxcols, use_w):
    CORES, D, DOUT = cfg["CORES"], cfg["D"], cfg["DOUT"]
    SHARD_PAD, NPAD, QROWS = cfg["SHARD_PAD"], cfg["NPAD"], cfg["QROWS"]
    NSB, SB_D, NBLOCKS, NQ = cfg["NSB"], cfg["SB_D"], cfg["NBLOCKS"], cfg["NQ"]
    Ctot = len(chunks)
    Cmax = max(r[2] for r in runs)

    nc = bacc.Bacc("TRN2", debug=False)
    ytab0 = nc.dram_tensor("ytab0", [NPAD, D], bf16, kind="ExternalInput")
    ytab_own = nc.dram_tensor("ytab_own", [SHARD_PAD, D], bf16, kind="ExternalInput")
    idx_in = nc.dram_tensor("idxr", [PB, idxcols], i16, kind="ExternalInput")
    dstv_in = nc.dram_tensor("dstvr", [PB, Ctot], f32, kind="ExternalInput")
    if use_w:
        wval_in = nc.dram_tensor("wvalr", [PB, Ctot], f32, kind="ExternalInput")
    disb_in = nc.dram_tensor("disb", [PB, SHARD_PAD], f32, kind="ExternalInput")
    sdis_in = nc.dram_tensor("selfdisb", [PB, SHARD_PAD], bf16, kind="ExternalInput")
    iota_in = nc.dram_tensor("iota", [PB, 2 * PB], bf16, kind="ExternalInput")
    w_in = [
        nc.dram_tensor("W0", [D, D], f32, kind="ExternalInput"),
        nc.dram_tensor("W1", [D, D], f32, kind="ExternalInput"),
        nc.dram_tensor("W2", [D, DOUT], f32, kind="ExternalInput"),
    ]
    b_in = [
        nc.dram_tensor("b0", [D, 1], f32, kind="ExternalInput"),
        nc.dram_tensor("b1", [D, 1], f32, kind="ExternalInput"),
        nc.dram_tensor("b2", [DOUT, 1], f32, kind="ExternalInput"),
    ]
    out_t = nc.dram_tensor("out_t", [DOUT, SHARD_PAD], f32, kind="ExternalOutput")
    ys = [nc.dram_tensor(f"ys{l}", [SHARD_PAD, D], bf16) for l in range(2)]
    yf = [
        nc.dram_tensor(f"yf{l}", [NPAD, D], bf16, addr_space="Shared")
        for l in range(2)
    ]

    with tile.TileContext(nc) as tc:
        with (
            tc.tile_pool(name="const", bufs=1) as constp,
            tc.tile_pool(name="gat", bufs=_BUFS.get("gat", 4)) as gatp,
            tc.tile_pool(name="ohp", bufs=_BUFS.get("oh", 10)) as ohp,
            tc.tile_pool(name="epi", bufs=_BUFS.get("epi", 2)) as epip,
            tc.tile_pool(name="psA", bufs=_BUFS.get("psA", 2), space="PSUM") as psA,
            tc.tile_pool(name="psB", bufs=1, space="PSUM") as psB,
        ):
            nc.gpsimd.load_library(library_config.mlp)
            idx_t = constp.tile([PB, idxcols], i16)
            dstv_t = constp.tile([PB, Ctot], f32)
            disb_t = constp.tile([PB, SHARD_PAD], f32)
            sdis_t = constp.tile([PB, SHARD_PAD], bf16)
            iota_t = constp.tile([PB, 2 * PB], bf16)
            ynx_all = constp.tile([PB, SHARD_PAD], bf16)
            nc.sync.dma_start(idx_t[:], idx_in[:])
            nc.sync.dma_start(dstv_t[:], dstv_in[:])
            if use_w:
                wval_t = constp.tile([PB, Ctot], f32)
                nc.sync.dma_start(wval_t[:], wval_in[:])
            nc.sync.dma_start(disb_t[:], disb_in[:])
            nc.sync.dma_start(sdis_t[:], sdis_in[:])
            nc.sync.dma_start(iota_t[:], iota_in[:])
            for i in range(SHARD_PAD // PB):
                nc.sync.dma_start(
                    ynx_all[:, i * PB : (i + 1) * PB],
                    ytab_own[i * PB : (i + 1) * PB, :],
                    transpose="notranspose" not in ABLATE,
                )
            w_t = []
            b_t = []
            for l in range(3):
                wt = constp.tile(list(w_in[l].shape), f32)
                bt = constp.tile(list(b_in[l].shape), f32)
                nc.sync.dma_start(wt[:], w_in[l][:])
                nc.sync.dma_start(bt[:], b_in[l][:])
                w_t.append(wt)
                b_t.append(bt)

            # group runs/chunks by sb
            run_by_sb = [[] for _ in range(NSB)]
            for ri, r in enumerate(runs):
                run_by_sb[r[0]].append(ri)
            chunk_of_run = [[] for _ in runs]
            for gc, ch in enumerate(chunks):
                # chunks are appended run-major in schedule order
                pass
            # recompute chunk->run mapping from order
            gc = 0
            for ri, r in enumerate(runs):
                for j in range(r[2]):
                    chunk_of_run[ri].append(gc)
                    gc += 1

            for layer in range(3):
                table = [ytab0, yf[0], yf[1]][layer]
                relu = layer < 2
                outp = D if layer < 2 else DOUT
                for s in range(NSB):
                    ncols = min(SB_D, NBLOCKS * PB - s * SB_D)
                    ps = psA.tile([PB, ncols], f32, tag="agg")
                    for ri in run_by_sb[s]:
                        _, qq, C, ioff = runs[ri]
                        gt = gatp.tile([PB, C, D], bf16, tag="gt")
                        nc.gpsimd.dma_gather(
                            gt[:],
                            table[qq * QROWS : (qq + 1) * QROWS, :],
                            idx_t[:, ioff : ioff + 8 * C],
                            PB * C,
                            PB * C,
                            D,
                            single_packet="singlepacket" in ABLATE,
                        )
                        for j, gc in enumerate(chunk_of_run[ri]):
                            _, _, wbase, flags = chunks[gc]
                            if flags[0] is None and flags[1] is None:
                                continue  # pure-pad chunk
                            lo = 0 if flags[0] is not None else PB
                            hi = 2 * PB if flags[1] is not None else PB
                            oh = ohp.tile([PB, 2 * PB], bf16, tag="oh")
                            if use_w:
                                nc.vector.tensor_scalar(
                                    oh[:, lo:hi],
                                    iota_t[:, lo:hi],
                                    dstv_t[:, gc : gc + 1],
                                    wval_t[:, gc : gc + 1],
                                    op0=mybir.AluOpType.is_equal,
                                    op1=mybir.AluOpType.mult,
                                )
                            else:
                                nc.vector.tensor_scalar(
                                    oh[:, lo:hi],
                                    iota_t[:, lo:hi],
                                    dstv_t[:, gc : gc + 1],
                                    None,
                                    op0=mybir.AluOpType.is_equal,
                                )
                            for h in range(2):
                                if flags[h] is None:
                                    continue
                                st, sp = flags[h]
                                nc.tensor.matmul(
                                    ps[:, wbase + h * PB : wbase + (h + 1) * PB],
                                    gt[:, j, :],
                                    oh[:, h * PB : (h + 1) * PB],
                                    start=st,
                                    stop=sp,
                                )
                    # epilogue for superblock s: rhs = agg*dis + y_own*(w_self*dis)
                    cols = slice(s * SB_D, s * SB_D + ncols)
                    u = epip.tile([PB, ncols], f32, tag="u")
                    nc.vector.tensor_tensor(
                        out=u[:], in0=ynx_all[:, cols], in1=sdis_t[:, cols],
                        op=mybir.AluOpType.mult,
                    )
                    rhs = epip.tile([PB, ncols], f32, tag="rhs")
                    nc.vector.tensor_tensor(
                        out=rhs[:], in0=ps[:], in1=disb_t[:, cols],
                        op=mybir.AluOpType.mult,
                    )
                    nc.vector.tensor_tensor(
                        out=rhs[:], in0=rhs[:], in1=u[:],
                        op=mybir.AluOpType.add,
                    )
                    zps = psB.tile([outp, ncols], f32, tag="z")
                    for half in range((ncols + 511) // 512):
                        hc = slice(half * 512, min(ncols, (half + 1) * 512))
                        nc.tensor.matmul(
                            zps[:, hc], w_t[layer][:], rhs[:, hc],
                            start=True, stop=True,
                        )
                    if layer < 2:
                        h_t = epip.tile([PB, ncols], f32, tag="h")
                        nc.scalar.activation(
                            h_t[:], zps[:],
                            mybir.ActivationFunctionType.Relu,
                            bias=b_t[layer][:], scale=1.0,
                        )
                        nc.vector.tensor_tensor(
                            out=ynx_all[:, cols], in0=h_t[:], in1=disb_t[:, cols],
                            op=mybir.AluOpType.mult,
                        )
                        ynode = epip.tile([PB, ncols // PB, PB], bf16, tag="ynode")
                        for i in range(ncols // PB):
                            nc.sync.dma_start(
                                ynode[:, i, :],
                                ynx_all[:, s * SB_D + i * PB : s * SB_D + (i + 1) * PB],
                                transpose="notranspose" not in ABLATE,
                            )
                        dview = ys[layer][
                            s * SB_D : s * SB_D + ncols, :
                        ].rearrange("(i p) f -> p i f", p=PB)
                        nc.sync.dma_start(dview, ynode[:])
                    else:
                        ot = epip.tile([DOUT, ncols], f32, tag="ot")
                        nc.vector.tensor_scalar(
                            ot[:], zps[:], b_t[layer][:], None,
                            op0=mybir.AluOpType.add,
                        )
                        nc.sync.dma_start(out_t[:, cols], ot[:])
                if layer < 2:
                    if "nocollective" in ABLATE:
                        nc.sync.dma_start(yf[layer][:SHARD_PAD, :], ys[layer][:])
                    else:
                        nc.gpsimd.collective_compute(
                            "AllGather",
                            mybir.AluOpType.bypass,
                            ins=[ys[layer][:]],
                            outs=[yf[layer][:]],
                            replica_groups=[list(range(CORES))],
                        )
    nc.compile()
    return nc


def prepare(x, edge_index, edge_weight, W0, b0, W1, b1, W2, b2, cfg):
    N, D, CORES, SHARD = cfg["N"], cfg["D"], cfg["CORES"], cfg["SHARD"]
    SHARD_PAD, NPAD = cfg["SHARD_PAD"], cfg["NPAD"]
    src = np.asarray(edge_index[0], np.int64)
    dst = np.asarray(edge_index[1], np.int64)
    ew = np.asarray(edge_weight, np.float32)
    x = np.asarray(x, np.float32)

    deg = np.bincount(dst, weights=ew.astype(np.float64), minlength=N)
    dis = np.where(deg > 0, 1.0 / np.sqrt(deg), 0.0).astype(np.float32)

    ytab = np.zeros((NPAD, D), ml_dtypes.bfloat16)
    pr = pad_row(np.arange(N), cfg)
    ytab[pr] = (dis[:, None] * x).astype(ml_dtypes.bfloat16)

    # self edges (src==dst) are applied analytically in the epilogue
    self_m = src == dst
    wself = np.bincount(
        dst[self_m], weights=ew[self_m].astype(np.float64), minlength=N
    ).astype(np.float32)
    nsrc, ndst, new = src[~self_m], dst[~self_m], ew[~self_m]
    use_w = not bool(np.all(new == 1.0))

    runs, chunks, idxr, dstv, wval = make_schedule(nsrc, ndst, new, cfg)

    iota = np.tile(np.arange(2 * PB, dtype=np.float32), (PB, 1)).astype(
        ml_dtypes.bfloat16
    )
    disb = np.zeros((CORES, PB, SHARD_PAD), np.float32)
    sdisb = np.zeros((CORES, PB, SHARD_PAD), ml_dtypes.bfloat16)
    for c in range(CORES):
        sh = slice(c * SHARD, (c + 1) * SHARD)
        disb[c, :, :SHARD] = dis[sh][None, :]
        sdisb[c, :, :SHARD] = (wself[sh] * dis[sh]).astype(ml_dtypes.bfloat16)[
            None, :
        ]

    shared = {
        "ytab0": ytab,
        "iota": iota,
        "W0": np.asarray(W0, np.float32),
        "W1": np.asarray(W1, np.float32),
        "W2": np.asarray(W2, np.float32),
        "b0": np.asarray(b0, np.float32).reshape(-1, 1),
        "b1": np.asarray(b1, np.float32).reshape(-1, 1),
        "b2": np.asarray(b2, np.float32).reshape(-1, 1),
    }
    in_maps = []
    for c in range(CORES):
        m = dict(shared)
        m["ytab_own"] = ytab[c * SHARD_PAD : (c + 1) * SHARD_PAD]
        m["idxr"] = idxr[c]
        m["dstvr"] = dstv[c]
        if use_w:
            m["wvalr"] = wval[c]
        m["disb"] = disb[c]
        m["selfdisb"] = sdisb[c]
        in_maps.append(m)
    return runs, chunks, in_maps, idxr.shape[2], use_w


def assemble(results, cfg):
    N, DOUT, CORES, SHARD = cfg["N"], cfg["DOUT"], cfg["CORES"], cfg["SHARD"]
    out = np.empty((N, DOUT), np.float32)
    for c in range(CORES):
        out[c * SHARD : (c + 1) * SHARD] = results[c]["out_t"][:, :SHARD].T
    return out


def run(inputs, cfg=None, trace=False, sim=False):
    cfg = cfg or make_cfg()
    runs, chunks, in_maps, idxcols, use_w = prepare(cfg=cfg, **inputs)
    nc = build_program(runs, chunks, cfg, idxcols, use_w)
    if sim:
        from concourse.bass_interp import MultiCoreSim

        msim = MultiCoreSim(nc, cfg["CORES"])
        for c in range(cfg["CORES"]):
            for k, v in in_maps[c].items():
                msim.cores[c].tensor(k)[:] = v
        msim.simulate()
        results = [
            {"out_t": np.asarray(msim.cores[c].tensor("out_t"))}
            for c in range(cfg["CORES"])
        ]
        return assemble(results, cfg), msim
    try:
        res = run_bass_kernel_spmd(
            nc, in_maps, list(range(cfg["CORES"])), trace=trace
        )
    except ModuleNotFoundError:
        # NTFF profiling hook unavailable in this container
        res = run_bass_kernel_spmd(nc, in_maps, list(range(cfg["CORES"])))
    return assemble(res.results, cfg), res


def kernel(**inputs):
    out, _ = run(inputs)
    return out



# revision 9
# speedup vs baseline: 1.0755x; 1.0755x over previous
"""3-layer GCN on 8 TRN2 NeuronCores (Bass/Tile).

Strategy (matches the sharding hint): nodes are partitioned across the 8
cores (12500 dst nodes each); each core owns the edges whose dst falls in
its shard. Per layer the core aggregates messages for its dst shard by
gathering source-node feature rows (dma_gather, bf16) and reducing them
into PSUM via one-hot matmuls on the TensorEngine; the dense transforms
(dis*agg @ W + b, relu) run on the shard. Node features for the next
layer are exchanged with an AllGather. GCN normalization is rewritten as

    h_{l+1} = relu( (dis ⊙ (A_w @ (dis ⊙ h_l))) @ W_l + b_l )

(A_w unweighted-except-edge-weight adjacency, dis = deg^-1/2), which is
exact because the dense transform commutes with the aggregation.
Edge weights ride inside the one-hot values (is_equal * w).

The per-(superblock, src-quadrant) chunk schedule is computed jointly
across all 8 cores so a single SPMD program fits every core; cores pad
their chunks (idx=0 rows with zero one-hot weight) where their edge
counts differ.
"""

import os
import sys

sys.path.insert(0, "/opt/trn_rl_repo")

ABLATE = set(os.environ.get("GCN_ABLATE", "").split(","))
_BUFS = {}
for kv in os.environ.get("GCN_BUFS", "").split(","):
    if "=" in kv:
        k, v = kv.split("=")
        _BUFS[k] = int(v)

import numpy as np
import ml_dtypes

import concourse.bacc as bacc
import concourse.mybir as mybir
import concourse.tile as tile
from concourse import library_config
from concourse.bass_utils import run_bass_kernel_spmd

f32 = mybir.dt.float32
bf16 = mybir.dt.bfloat16
i16 = mybir.dt.int16

PB = 128  # partition / block size


def make_cfg(N=100000, D=128, DOUT=64, CORES=8, SB_BLOCKS=8, NQ=4):
    shard = N // CORES
    assert shard * CORES == N
    shard_pad = ((shard + PB - 1) // PB) * PB
    npad = shard_pad * CORES
    assert npad % NQ == 0
    qrows = npad // NQ
    assert qrows <= 32768, "dma_gather int16 index range"
    nblocks = shard_pad // PB
    sb_d = SB_BLOCKS * PB
    nsb = (nblocks + SB_BLOCKS - 1) // SB_BLOCKS
    return dict(
        N=N, D=D, DOUT=DOUT, CORES=CORES, SHARD=shard, SHARD_PAD=shard_pad,
        NPAD=npad, NQ=NQ, QROWS=qrows, NBLOCKS=nblocks, SB_D=sb_d, NSB=nsb,
    )


def pad_row(n, cfg):
    return (n // cfg["SHARD"]) * cfg["SHARD_PAD"] + n % cfg["SHARD"]


def make_schedule(edge_src, edge_dst, edge_w, cfg):
    """Cross-core-uniform chunk schedule.

    Returns (runs, chunks, per-core arrays):
      runs: list of (sb, q, C, idx_off) in emission order
      chunks: list of (sb, q, wbase, flags) where flags = ((start0, stop0),
              (start1, stop1)) for the two 128-wide matmul halves
      idxr [CORES, 128, idxcols] i16, dstv/wval [CORES, 128, Ctot] f32
    """
    CORES, SHARD, SB_D = cfg["CORES"], cfg["SHARD"], cfg["SB_D"]
    QROWS, NSB, NBLOCKS = cfg["QROWS"], cfg["NSB"], cfg["NBLOCKS"]
    NQ = cfg["NQ"]

    per = []  # [c][sb][q] -> (dsl, idxq, wv) sorted by dsl
    for c in range(CORES):
        m = (edge_dst >= c * SHARD) & (edge_dst < (c + 1) * SHARD)
        dl = edge_dst[m] - c * SHARD
        sp = pad_row(edge_src[m], cfg)
        wv = edge_w[m]
        sb = dl // SB_D
        q = sp // QROWS
        order = np.lexsort((dl, q, sb))
        dl, sp, wv, sb, q = dl[order], sp[order], wv[order], sb[order], q[order]
        key = sb * NQ + q
        bounds = np.searchsorted(key, np.arange(NSB * NQ + 1))
        rows = []
        for s in range(NSB):
            qs = []
            for qq in range(NQ):
                lo, hi = bounds[s * NQ + qq], bounds[s * NQ + qq + 1]
                qs.append((
                    (dl[lo:hi] - s * SB_D).astype(np.int32),
                    (sp[lo:hi] - qq * QROWS).astype(np.int32),
                    wv[lo:hi].astype(np.float32),
                ))
            rows.append(qs)
        per.append(rows)

    runs = []
    chunks = []
    ch_dstv, ch_wval = [], []   # per chunk: [CORES,128] arrays
    run_idx_blocks = []         # per run: [CORES, 128, 8*C] i16
    first_mm = {}               # (sb, bank) -> chunk half getting start
    last_mm = {}
    idx_off = 0
    for s in range(NSB):
        ncols = min(SB_D, NBLOCKS * PB - s * SB_D)
        for qq in range(NQ):
            datas = [per[c][s][qq] for c in range(CORES)]
            lens = [d[0].shape[0] for d in datas]
            if max(lens) == 0:
                continue
            ptr = [0] * CORES
            run_chunk_idx = []  # [CORES,128] per chunk
            C = 0
            while True:
                active = [c for c in range(CORES) if ptr[c] < lens[c]]
                if not active:
                    break
                mind = min(int(datas[c][0][ptr[c]]) for c in active)
                wbase = min((mind // PB) * PB, max(0, ncols - 2 * PB))
                dv = np.full((CORES, PB), -1.0, np.float32)
                wv = np.zeros((CORES, PB), np.float32)
                ix = np.zeros((CORES, PB), np.int16)
                for c in range(CORES):
                    dl, iq, ww = datas[c]
                    p = ptr[c]
                    hi = np.searchsorted(dl, wbase + 2 * PB, side="left")
                    take = min(PB, hi - p)
                    if take > 0:
                        dv[c, :take] = dl[p : p + take] - wbase
                        wv[c, :take] = ww[p : p + take]
                        ix[c, :take] = iq[p : p + take]
                        ptr[c] = p + take
                gc = len(chunks)
                used = [
                    bool(((dv >= 0) & (dv < PB)).any()),
                    bool((dv >= PB).any()),
                ]
                flags = []
                for h in range(2):
                    col = wbase + h * PB
                    if col >= ncols or not used[h]:
                        flags.append(None)
                        continue
                    bank = (s, col // 512)
                    st = bank not in first_mm
                    if st:
                        first_mm[bank] = (gc, h)
                    last_mm[bank] = (gc, h)
                    flags.append(st)
                chunks.append([s, qq, wbase, flags])
                ch_dstv.append(dv)
                ch_wval.append(wv)
                run_chunk_idx.append(ix)
                C += 1
                if C > (max(lens) // PB) + NBLOCKS + 16:
                    raise RuntimeError("chunk packing did not converge")
            # idx region for the run: flat [128*C] -> [16, 8C] -> [128, 8C]
            blk = np.zeros((CORES, PB, 8 * C), np.int16)
            for c in range(CORES):
                flat = np.concatenate([ci[c] for ci in run_chunk_idx])
                wrapped = flat.reshape(-1, 16).T  # [16, 8C]
                blk[c] = np.tile(wrapped, (8, 1))
            run_idx_blocks.append(blk)
            runs.append([s, qq, C, idx_off])
            idx_off += 8 * C

    # every (sb, bank) must receive at least one matmul (else stale PSUM)
    for s in range(NSB):
        ncols = min(SB_D, NBLOCKS * PB - s * SB_D)
        for bank in range((ncols + 511) // 512):
            assert (s, bank) in first_mm, f"uncovered psum bank {(s, bank)}"

    # stop flags
    stops = {v: k for k, v in last_mm.items()}
    for gc, ch in enumerate(chunks):
        fl = ch[3]
        ch[3] = tuple(
            None if fl[h] is None else (fl[h], (gc, h) in stops) for h in range(2)
        )

    Ctot = len(chunks)
    dstv = np.stack(ch_dstv, axis=2)  # [CORES, 128, Ctot]
    wval = np.stack(ch_wval, axis=2)
    idxr = np.concatenate(run_idx_blocks, axis=2)  # [CORES, 128, idxcols]
    return runs, chunks, idxr, dstv, wval


def build_program(runs, chunks, cfg, idxcols, use_w):
    CORES, D, DOUT = cfg["CORES"], cfg["D"], cfg["DOUT"]
    SHARD_PAD, NPAD, QROWS = cfg["SHARD_PAD"], cfg["NPAD"], cfg["QROWS"]
    NSB, SB_D, NBLOCKS, NQ = cfg["NSB"], cfg["SB_D"], cfg["NBLOCKS"], cfg["NQ"]
    Ctot = len(chunks)
    Cmax = max(r[2] for r in runs)

    nc = bacc.Bacc("TRN2", debug=False)
    ytab0 = nc.dram_tensor("ytab0", [NPAD, D], bf16, kind="ExternalInput")
    ytab_own = nc.dram_tensor("ytab_own", [SHARD_PAD, D], bf16, kind="ExternalInput")
    idx_in = nc.dram_tensor("idxr", [PB, idxcols], i16, kind="ExternalInput")
    dstv_in = nc.dram_tensor("dstvr", [PB, Ctot], f32, kind="ExternalInput")
    if use_w:
        wval_in = nc.dram_tensor("wvalr", [PB, Ctot], f32, kind="ExternalInput")
    sdis_in = nc.dram_tensor("selfdisb", [PB, SHARD_PAD], bf16, kind="ExternalInput")
    iota_in = nc.dram_tensor("iota", [PB, 2 * PB], bf16, kind="ExternalInput")
    w_in = [
        nc.dram_tensor("W0", [D, D], f32, kind="ExternalInput"),
        nc.dram_tensor("W1", [D, D], f32, kind="ExternalInput"),
        nc.dram_tensor("W2", [D, DOUT], f32, kind="ExternalInput"),
    ]
    b_in = [
        nc.dram_tensor("b0", [D, 1], f32, kind="ExternalInput"),
        nc.dram_tensor("b1", [D, 1], f32, kind="ExternalInput"),
        nc.dram_tensor("b2", [DOUT, 1], f32, kind="ExternalInput"),
    ]
    out_t = nc.dram_tensor("out_t", [DOUT, SHARD_PAD], f32, kind="ExternalOutput")
    ys = [nc.dram_tensor(f"ys{l}", [SHARD_PAD, D], bf16) for l in range(2)]
    yf = [
        nc.dram_tensor(f"yf{l}", [NPAD, D], bf16, addr_space="Shared")
        for l in range(2)
    ]

    with tile.TileContext(nc) as tc:
        with (
            tc.tile_pool(name="const", bufs=1) as constp,
            tc.tile_pool(name="gat", bufs=_BUFS.get("gat", 4)) as gatp,
            tc.tile_pool(name="ohp", bufs=_BUFS.get("oh", 10)) as ohp,
            tc.tile_pool(name="epi", bufs=_BUFS.get("epi", 2)) as epip,
            tc.tile_pool(name="psA", bufs=_BUFS.get("psA", 2), space="PSUM") as psA,
            tc.tile_pool(name="psB", bufs=1, space="PSUM") as psB,
        ):
            nc.gpsimd.load_library(library_config.mlp)
            idx_t = constp.tile([PB, idxcols], i16)
            dstv_t = constp.tile([PB, Ctot], f32)
            sdis_t = constp.tile([PB, SHARD_PAD], bf16)
            iota_t = constp.tile([PB, 2 * PB], bf16)
            ynx_all = constp.tile([PB, SHARD_PAD], bf16)
            nc.sync.dma_start(idx_t[:], idx_in[:])
            nc.sync.dma_start(dstv_t[:], dstv_in[:])
            if use_w:
                wval_t = constp.tile([PB, Ctot], f32)
                nc.sync.dma_start(wval_t[:], wval_in[:])
            nc.sync.dma_start(sdis_t[:], sdis_in[:])
            nc.sync.dma_start(iota_t[:], iota_in[:])
            for i in range(SHARD_PAD // PB):
                nc.sync.dma_start(
                    ynx_all[:, i * PB : (i + 1) * PB],
                    ytab_own[i * PB : (i + 1) * PB, :],
                    transpose="notranspose" not in ABLATE,
                )
            w_t = []
            b_t = []
            for l in range(3):
                wt = constp.tile(list(w_in[l].shape), f32)
                bt = constp.tile(list(b_in[l].shape), f32)
                nc.sync.dma_start(wt[:], w_in[l][:])
                nc.sync.dma_start(bt[:], b_in[l][:])
                w_t.append(wt)
                b_t.append(bt)

            # group runs/chunks by sb
            run_by_sb = [[] for _ in range(NSB)]
            for ri, r in enumerate(runs):
                run_by_sb[r[0]].append(ri)
            chunk_of_run = [[] for _ in runs]
            for gc, ch in enumerate(chunks):
                # chunks are appended run-major in schedule order
                pass
            # recompute chunk->run mapping from order
            gc = 0
            for ri, r in enumerate(runs):
                for j in range(r[2]):
                    chunk_of_run[ri].append(gc)
                    gc += 1

            for layer in range(3):
                table = [ytab0, yf[0], yf[1]][layer]
                relu = layer < 2
                outp = D if layer < 2 else DOUT
                for s in range(NSB):
                    ncols = min(SB_D, NBLOCKS * PB - s * SB_D)
                    ps = psA.tile([PB, ncols], f32, tag="agg")
                    for ri in run_by_sb[s]:
                        _, qq, C, ioff = runs[ri]
                        gt = gatp.tile([PB, C, D], bf16, tag="gt")
                        nc.gpsimd.dma_gather(
                            gt[:],
                            table[qq * QROWS : (qq + 1) * QROWS, :],
                            idx_t[:, ioff : ioff + 8 * C],
                            PB * C,
                            PB * C,
                            D,
                            single_packet="singlepacket" in ABLATE,
                        )
                        for j, gc in enumerate(chunk_of_run[ri]):
                            _, _, wbase, flags = chunks[gc]
                            if flags[0] is None and flags[1] is None:
                                continue  # pure-pad chunk
                            lo = 0 if flags[0] is not None else PB
                            hi = 2 * PB if flags[1] is not None else PB
                            oh = ohp.tile([PB, 2 * PB], bf16, tag="oh")
                            if use_w:
                                nc.vector.tensor_scalar(
                                    oh[:, lo:hi],
                                    iota_t[:, lo:hi],
                                    dstv_t[:, gc : gc + 1],
                                    wval_t[:, gc : gc + 1],
                                    op0=mybir.AluOpType.is_equal,
                                    op1=mybir.AluOpType.mult,
                                )
                            else:
                                nc.vector.tensor_scalar(
                                    oh[:, lo:hi],
                                    iota_t[:, lo:hi],
                                    dstv_t[:, gc : gc + 1],
                                    None,
                                    op0=mybir.AluOpType.is_equal,
                                )
                            for h in range(2):
                                if flags[h] is None:
                                    continue
                                st, sp = flags[h]
                                nc.tensor.matmul(
                                    ps[:, wbase + h * PB : wbase + (h + 1) * PB],
                                    gt[:, j, :],
                                    oh[:, h * PB : (h + 1) * PB],
                                    start=st,
                                    stop=sp,
                                )
                    # epilogue for superblock s: rhs = agg + y_own*(w_self*dis^2)
                    # (edge-side dis norms are folded into the one-hot weights)
                    cols = slice(s * SB_D, s * SB_D + ncols)
                    u = epip.tile([PB, ncols], bf16, tag="u")
                    nc.vector.tensor_tensor(
                        out=u[:], in0=ynx_all[:, cols], in1=sdis_t[:, cols],
                        op=mybir.AluOpType.mult,
                    )
                    rhs = epip.tile([PB, ncols], f32, tag="rhs")
                    nc.vector.tensor_tensor(
                        out=rhs[:], in0=ps[:], in1=u[:],
                        op=mybir.AluOpType.add,
                    )
                    zps = psB.tile([outp, ncols], f32, tag="z")
                    for half in range((ncols + 511) // 512):
                        hc = slice(half * 512, min(ncols, (half + 1) * 512))
                        nc.tensor.matmul(
                            zps[:, hc], w_t[layer][:], rhs[:, hc],
                            start=True, stop=True,
                        )
                    if layer < 2:
                        nc.scalar.activation(
                            ynx_all[:, cols], zps[:],
                            mybir.ActivationFunctionType.Relu,
                            bias=b_t[layer][:], scale=1.0,
                        )
                        ynode = epip.tile([PB, ncols // PB, PB], bf16, tag="ynode")
                        for i in range(ncols // PB):
                            nc.sync.dma_start(
                                ynode[:, i, :],
                                ynx_all[:, s * SB_D + i * PB : s * SB_D + (i + 1) * PB],
                                transpose="notranspose" not in ABLATE,
                            )
                        dview = ys[layer][
                            s * SB_D : s * SB_D + ncols, :
                        ].rearrange("(i p) f -> p i f", p=PB)
                        nc.sync.dma_start(dview, ynode[:])
                    else:
                        ot = epip.tile([DOUT, ncols], f32, tag="ot")
                        nc.vector.tensor_scalar(
                            ot[:], zps[:], b_t[layer][:], None,
                            op0=mybir.AluOpType.add,
                        )
                        nc.sync.dma_start(out_t[:, cols], ot[:])
                if layer < 2:
                    if "nocollective" in ABLATE:
                        nc.sync.dma_start(yf[layer][:SHARD_PAD, :], ys[layer][:])
                    else:
                        nc.gpsimd.collective_compute(
                            "AllGather",
                            mybir.AluOpType.bypass,
                            ins=[ys[layer][:]],
                            outs=[yf[layer][:]],
                            replica_groups=[list(range(CORES))],
                        )
    nc.compile()
    return nc


def prepare(x, edge_index, edge_weight, W0, b0, W1, b1, W2, b2, cfg):
    N, D, CORES, SHARD = cfg["N"], cfg["D"], cfg["CORES"], cfg["SHARD"]
    SHARD_PAD, NPAD = cfg["SHARD_PAD"], cfg["NPAD"]
    src = np.asarray(edge_index[0], np.int64)
    dst = np.asarray(edge_index[1], np.int64)
    ew = np.asarray(edge_weight, np.float32)
    x = np.asarray(x, np.float32)

    deg = np.bincount(dst, weights=ew.astype(np.float64), minlength=N)
    dis = np.where(deg > 0, 1.0 / np.sqrt(deg), 0.0).astype(np.float32)

    # normalization folded into the edge weights: table holds raw h
    ytab = np.zeros((NPAD, D), ml_dtypes.bfloat16)
    pr = pad_row(np.arange(N), cfg)
    ytab[pr] = x.astype(ml_dtypes.bfloat16)

    # self edges (src==dst) are applied analytically in the epilogue
    self_m = src == dst
    wself = np.bincount(
        dst[self_m], weights=ew[self_m].astype(np.float64), minlength=N
    ).astype(np.float32)
    nsrc, ndst, new = src[~self_m], dst[~self_m], ew[~self_m]
    new = new * dis[ndst] * dis[nsrc]
    use_w = True

    runs, chunks, idxr, dstv, wval = make_schedule(nsrc, ndst, new, cfg)

    iota = np.tile(np.arange(2 * PB, dtype=np.float32), (PB, 1)).astype(
        ml_dtypes.bfloat16
    )
    sdisb = np.zeros((CORES, PB, SHARD_PAD), ml_dtypes.bfloat16)
    for c in range(CORES):
        sh = slice(c * SHARD, (c + 1) * SHARD)
        sdisb[c, :, :SHARD] = (wself[sh] * dis[sh] * dis[sh]).astype(
            ml_dtypes.bfloat16
        )[None, :]

    shared = {
        "ytab0": ytab,
        "iota": iota,
        "W0": np.asarray(W0, np.float32),
        "W1": np.asarray(W1, np.float32),
        "W2": np.asarray(W2, np.float32),
        "b0": np.asarray(b0, np.float32).reshape(-1, 1),
        "b1": np.asarray(b1, np.float32).reshape(-1, 1),
        "b2": np.asarray(b2, np.float32).reshape(-1, 1),
    }
    in_maps = []
    for c in range(CORES):
        m = dict(shared)
        m["ytab_own"] = ytab[c * SHARD_PAD : (c + 1) * SHARD_PAD]
        m["idxr"] = idxr[c]
        m["dstvr"] = dstv[c]
        if use_w:
            m["wvalr"] = wval[c]
        m["selfdisb"] = sdisb[c]
        in_maps.append(m)
    return runs, chunks, in_maps, idxr.shape[2], use_w


def assemble(results, cfg):
    N, DOUT, CORES, SHARD = cfg["N"], cfg["DOUT"], cfg["CORES"], cfg["SHARD"]
    out = np.empty((N, DOUT), np.float32)
    for c in range(CORES):
        out[c * SHARD : (c + 1) * SHARD] = results[c]["out_t"][:, :SHARD].T
    return out


def run(inputs, cfg=None, trace=False, sim=False):
    cfg = cfg or make_cfg()
    runs, chunks, in_maps, idxcols, use_w = prepare(cfg=cfg, **inputs)
    nc = build_program(runs, chunks, cfg, idxcols, use_w)
    if sim:
        from concourse.bass_interp import MultiCoreSim

        msim = MultiCoreSim(nc, cfg["CORES"])
        for c in range(cfg["CORES"]):
            for k, v in in_maps[c].items():
                msim.cores[c].tensor(k)[:] = v
        msim.simulate()
        results = [
            {"out_t": np.asarray(msim.cores[c].tensor("out_t"))}
            for c in range(cfg["CORES"])
        ]
        return assemble(results, cfg), msim
    try:
        res = run_bass_kernel_spmd(
            nc, in_maps, list(range(cfg["CORES"])), trace=trace
        )
    except ModuleNotFoundError:
        # NTFF profiling hook unavailable in this container
        res = run_bass_kernel_spmd(nc, in_maps, list(range(cfg["CORES"])))
    return assemble(res.results, cfg), res


def kernel(**inputs):
    out, _ = run(inputs)
    return out



# revision 11
# speedup vs baseline: 1.0790x; 1.0033x over previous
"""3-layer GCN on 8 TRN2 NeuronCores (Bass/Tile).

Strategy (matches the sharding hint): nodes are partitioned across the 8
cores (12500 dst nodes each); each core owns the edges whose dst falls in
its shard. Per layer the core aggregates messages for its dst shard by
gathering source-node feature rows (dma_gather, bf16) and reducing them
into PSUM via one-hot matmuls on the TensorEngine; the dense transforms
(agg @ W + b, relu) run on the shard. Node features for the next layer
are exchanged with an AllGather. The full GCN normalization
dis[dst]*w*dis[src] (dis = deg^-1/2) is folded into the per-edge one-hot
values host-side (is_equal * wval), so the table holds raw h and the
epilogue needs only the self-loop term rhs = agg + h_own*(w_self*dis^2);
this is exact because the dense transform commutes with aggregation.

The per-(superblock, src-quadrant) chunk schedule is computed jointly
across all 8 cores so a single SPMD program fits every core; cores pad
their chunks (idx=0 rows with zero one-hot weight) where their edge
counts differ.
"""

import os
import sys

sys.path.insert(0, "/opt/trn_rl_repo")

ABLATE = set(os.environ.get("GCN_ABLATE", "").split(","))
_BUFS = {}
for kv in os.environ.get("GCN_BUFS", "").split(","):
    if "=" in kv:
        k, v = kv.split("=")
        _BUFS[k] = int(v)

import numpy as np
import ml_dtypes

import concourse.bacc as bacc
import concourse.mybir as mybir
import concourse.tile as tile
from concourse import library_config
from concourse.bass_utils import run_bass_kernel_spmd

f32 = mybir.dt.float32
bf16 = mybir.dt.bfloat16
i16 = mybir.dt.int16

PB = 128  # partition / block size


def make_cfg(N=100000, D=128, DOUT=64, CORES=8, SB_BLOCKS=8, NQ=4):
    shard = N // CORES
    assert shard * CORES == N
    shard_pad = ((shard + PB - 1) // PB) * PB
    npad = shard_pad * CORES
    assert npad % NQ == 0
    qrows = npad // NQ
    assert qrows <= 32768, "dma_gather int16 index range"
    nblocks = shard_pad // PB
    sb_d = SB_BLOCKS * PB
    nsb = (nblocks + SB_BLOCKS - 1) // SB_BLOCKS
    return dict(
        N=N, D=D, DOUT=DOUT, CORES=CORES, SHARD=shard, SHARD_PAD=shard_pad,
        NPAD=npad, NQ=NQ, QROWS=qrows, NBLOCKS=nblocks, SB_D=sb_d, NSB=nsb,
    )


def pad_row(n, cfg):
    return (n // cfg["SHARD"]) * cfg["SHARD_PAD"] + n % cfg["SHARD"]


def make_schedule(edge_src, edge_dst, edge_w, cfg):
    """Cross-core-uniform chunk schedule.

    Returns (runs, chunks, per-core arrays):
      runs: list of (sb, q, C, idx_off) in emission order
      chunks: list of (sb, q, wbase, flags) where flags = ((start0, stop0),
              (start1, stop1)) for the two 128-wide matmul halves
      idxr [CORES, 128, idxcols] i16, dstv/wval [CORES, 128, Ctot] f32
    """
    CORES, SHARD, SB_D = cfg["CORES"], cfg["SHARD"], cfg["SB_D"]
    QROWS, NSB, NBLOCKS = cfg["QROWS"], cfg["NSB"], cfg["NBLOCKS"]
    NQ = cfg["NQ"]

    per = []  # [c][sb][q] -> (dsl, idxq, wv) sorted by dsl
    for c in range(CORES):
        m = (edge_dst >= c * SHARD) & (edge_dst < (c + 1) * SHARD)
        dl = edge_dst[m] - c * SHARD
        sp = pad_row(edge_src[m], cfg)
        wv = edge_w[m]
        sb = dl // SB_D
        q = sp // QROWS
        order = np.lexsort((dl, q, sb))
        dl, sp, wv, sb, q = dl[order], sp[order], wv[order], sb[order], q[order]
        key = sb * NQ + q
        bounds = np.searchsorted(key, np.arange(NSB * NQ + 1))
        rows = []
        for s in range(NSB):
            qs = []
            for qq in range(NQ):
                lo, hi = bounds[s * NQ + qq], bounds[s * NQ + qq + 1]
                qs.append((
                    (dl[lo:hi] - s * SB_D).astype(np.int32),
                    (sp[lo:hi] - qq * QROWS).astype(np.int32),
                    wv[lo:hi].astype(np.float32),
                ))
            rows.append(qs)
        per.append(rows)

    runs = []
    chunks = []
    ch_dstv, ch_wval = [], []   # per chunk: [CORES,128] arrays
    run_idx_blocks = []         # per run: [CORES, 128, 8*C] i16
    first_mm = {}               # (sb, bank) -> chunk half getting start
    last_mm = {}
    idx_off = 0
    for s in range(NSB):
        ncols = min(SB_D, NBLOCKS * PB - s * SB_D)
        for qq in range(NQ):
            datas = [per[c][s][qq] for c in range(CORES)]
            lens = [d[0].shape[0] for d in datas]
            if max(lens) == 0:
                continue
            ptr = [0] * CORES
            run_chunk_idx = []  # [CORES,128] per chunk
            C = 0
            while True:
                active = [c for c in range(CORES) if ptr[c] < lens[c]]
                if not active:
                    break
                mind = min(int(datas[c][0][ptr[c]]) for c in active)
                wbase = min((mind // PB) * PB, max(0, ncols - 2 * PB))
                dv = np.full((CORES, PB), -1.0, np.float32)
                wv = np.zeros((CORES, PB), np.float32)
                ix = np.zeros((CORES, PB), np.int16)
                for c in range(CORES):
                    dl, iq, ww = datas[c]
                    p = ptr[c]
                    hi = np.searchsorted(dl, wbase + 2 * PB, side="left")
                    take = min(PB, hi - p)
                    if take > 0:
                        dv[c, :take] = dl[p : p + take] - wbase
                        wv[c, :take] = ww[p : p + take]
                        ix[c, :take] = iq[p : p + take]
                        ptr[c] = p + take
                gc = len(chunks)
                used = [
                    bool(((dv >= 0) & (dv < PB)).any()),
                    bool((dv >= PB).any()),
                ]
                flags = []
                for h in range(2):
                    col = wbase + h * PB
                    if col >= ncols or not used[h]:
                        flags.append(None)
                        continue
                    bank = (s, col // 512)
                    st = bank not in first_mm
                    if st:
                        first_mm[bank] = (gc, h)
                    last_mm[bank] = (gc, h)
                    flags.append(st)
                chunks.append([s, qq, wbase, flags])
                ch_dstv.append(dv)
                ch_wval.append(wv)
                run_chunk_idx.append(ix)
                C += 1
                if C > (max(lens) // PB) + NBLOCKS + 16:
                    raise RuntimeError("chunk packing did not converge")
            # idx region for the run: flat [128*C] -> [16, 8C] -> [128, 8C]
            blk = np.zeros((CORES, PB, 8 * C), np.int16)
            for c in range(CORES):
                flat = np.concatenate([ci[c] for ci in run_chunk_idx])
                wrapped = flat.reshape(-1, 16).T  # [16, 8C]
                blk[c] = np.tile(wrapped, (8, 1))
            run_idx_blocks.append(blk)
            runs.append([s, qq, C, idx_off])
            idx_off += 8 * C

    # every (sb, bank) must receive at least one matmul (else stale PSUM)
    for s in range(NSB):
        ncols = min(SB_D, NBLOCKS * PB - s * SB_D)
        for bank in range((ncols + 511) // 512):
            assert (s, bank) in first_mm, f"uncovered psum bank {(s, bank)}"

    # stop flags
    stops = {v: k for k, v in last_mm.items()}
    for gc, ch in enumerate(chunks):
        fl = ch[3]
        ch[3] = tuple(
            None if fl[h] is None else (fl[h], (gc, h) in stops) for h in range(2)
        )

    Ctot = len(chunks)
    dstv = np.stack(ch_dstv, axis=2)  # [CORES, 128, Ctot]
    wval = np.stack(ch_wval, axis=2)
    idxr = np.concatenate(run_idx_blocks, axis=2)  # [CORES, 128, idxcols]
    return runs, chunks, idxr, dstv, wval


def build_program(runs, chunks, cfg, idxcols, use_w):
    CORES, D, DOUT = cfg["CORES"], cfg["D"], cfg["DOUT"]
    SHARD_PAD, NPAD, QROWS = cfg["SHARD_PAD"], cfg["NPAD"], cfg["QROWS"]
    NSB, SB_D, NBLOCKS, NQ = cfg["NSB"], cfg["SB_D"], cfg["NBLOCKS"], cfg["NQ"]
    Ctot = len(chunks)
    Cmax = max(r[2] for r in runs)

    nc = bacc.Bacc("TRN2", debug=False)
    ytab0 = nc.dram_tensor("ytab0", [NPAD, D], bf16, kind="ExternalInput")
    ytab_own = nc.dram_tensor("ytab_own", [SHARD_PAD, D], bf16, kind="ExternalInput")
    idx_in = nc.dram_tensor("idxr", [PB, idxcols], i16, kind="ExternalInput")
    dstv_in = nc.dram_tensor("dstvr", [PB, Ctot], f32, kind="ExternalInput")
    if use_w:
        wval_in = nc.dram_tensor("wvalr", [PB, Ctot], f32, kind="ExternalInput")
    sdis_in = nc.dram_tensor("selfdisb", [PB, SHARD_PAD], bf16, kind="ExternalInput")
    iota_in = nc.dram_tensor("iota", [PB, 2 * PB], bf16, kind="ExternalInput")
    w_in = [
        nc.dram_tensor("W0", [D, D], f32, kind="ExternalInput"),
        nc.dram_tensor("W1", [D, D], f32, kind="ExternalInput"),
        nc.dram_tensor("W2", [D, DOUT], f32, kind="ExternalInput"),
    ]
    b_in = [
        nc.dram_tensor("b0", [D, 1], f32, kind="ExternalInput"),
        nc.dram_tensor("b1", [D, 1], f32, kind="ExternalInput"),
        nc.dram_tensor("b2", [DOUT, 1], f32, kind="ExternalInput"),
    ]
    out_t = nc.dram_tensor("out_t", [DOUT, SHARD_PAD], f32, kind="ExternalOutput")
    ys = [nc.dram_tensor(f"ys{l}", [SHARD_PAD, D], bf16) for l in range(2)]
    yf = [
        nc.dram_tensor(f"yf{l}", [NPAD, D], bf16, addr_space="Shared")
        for l in range(2)
    ]

    with tile.TileContext(nc) as tc:
        with (
            tc.tile_pool(name="const", bufs=1) as constp,
            tc.tile_pool(name="gat", bufs=_BUFS.get("gat", 6)) as gatp,
            tc.tile_pool(name="ohp", bufs=_BUFS.get("oh", 14)) as ohp,
            tc.tile_pool(name="epi", bufs=_BUFS.get("epi", 3)) as epip,
            tc.tile_pool(name="psA", bufs=_BUFS.get("psA", 2), space="PSUM") as psA,
            tc.tile_pool(name="psB", bufs=1, space="PSUM") as psB,
        ):
            nc.gpsimd.load_library(library_config.mlp)
            idx_t = constp.tile([PB, idxcols], i16)
            dstv_t = constp.tile([PB, Ctot], f32)
            sdis_t = constp.tile([PB, SHARD_PAD], bf16)
            iota_t = constp.tile([PB, 2 * PB], bf16)
            ynx_all = constp.tile([PB, SHARD_PAD], bf16)
            nc.sync.dma_start(idx_t[:], idx_in[:])
            nc.sync.dma_start(dstv_t[:], dstv_in[:])
            if use_w:
                wval_t = constp.tile([PB, Ctot], f32)
                nc.sync.dma_start(wval_t[:], wval_in[:])
            nc.sync.dma_start(sdis_t[:], sdis_in[:])
            nc.sync.dma_start(iota_t[:], iota_in[:])
            for i in range(SHARD_PAD // PB):
                nc.sync.dma_start(
                    ynx_all[:, i * PB : (i + 1) * PB],
                    ytab_own[i * PB : (i + 1) * PB, :],
                    transpose="notranspose" not in ABLATE,
                )
            w_t = []
            b_t = []
            for l in range(3):
                wt = constp.tile(list(w_in[l].shape), f32)
                bt = constp.tile(list(b_in[l].shape), f32)
                nc.sync.dma_start(wt[:], w_in[l][:])
                nc.sync.dma_start(bt[:], b_in[l][:])
                w_t.append(wt)
                b_t.append(bt)

            # group runs/chunks by sb
            run_by_sb = [[] for _ in range(NSB)]
            for ri, r in enumerate(runs):
                run_by_sb[r[0]].append(ri)
            chunk_of_run = [[] for _ in runs]
            for gc, ch in enumerate(chunks):
                # chunks are appended run-major in schedule order
                pass
            # recompute chunk->run mapping from order
            gc = 0
            for ri, r in enumerate(runs):
                for j in range(r[2]):
                    chunk_of_run[ri].append(gc)
                    gc += 1

            for layer in range(3):
                table = [ytab0, yf[0], yf[1]][layer]
                relu = layer < 2
                outp = D if layer < 2 else DOUT
                for s in range(NSB):
                    ncols = min(SB_D, NBLOCKS * PB - s * SB_D)
                    ps = psA.tile([PB, ncols], f32, tag="agg")
                    for ri in run_by_sb[s]:
                        _, qq, C, ioff = runs[ri]
                        gt = gatp.tile([PB, C, D], bf16, tag="gt")
                        nc.gpsimd.dma_gather(
                            gt[:],
                            table[qq * QROWS : (qq + 1) * QROWS, :],
                            idx_t[:, ioff : ioff + 8 * C],
                            PB * C,
                            PB * C,
                            D,
                            single_packet="singlepacket" in ABLATE,
                        )
                        for j, gc in enumerate(chunk_of_run[ri]):
                            _, _, wbase, flags = chunks[gc]
                            if flags[0] is None and flags[1] is None:
                                continue  # pure-pad chunk
                            lo = 0 if flags[0] is not None else PB
                            hi = 2 * PB if flags[1] is not None else PB
                            oh = ohp.tile([PB, 2 * PB], bf16, tag="oh")
                            if use_w:
                                nc.vector.tensor_scalar(
                                    oh[:, lo:hi],
                                    iota_t[:, lo:hi],
                                    dstv_t[:, gc : gc + 1],
                                    wval_t[:, gc : gc + 1],
                                    op0=mybir.AluOpType.is_equal,
                                    op1=mybir.AluOpType.mult,
                                )
                            else:
                                nc.vector.tensor_scalar(
                                    oh[:, lo:hi],
                                    iota_t[:, lo:hi],
                                    dstv_t[:, gc : gc + 1],
                                    None,
                                    op0=mybir.AluOpType.is_equal,
                                )
                            for h in range(2):
                                if flags[h] is None:
                                    continue
                                st, sp = flags[h]
                                nc.tensor.matmul(
                                    ps[:, wbase + h * PB : wbase + (h + 1) * PB],
                                    gt[:, j, :],
                                    oh[:, h * PB : (h + 1) * PB],
                                    start=st,
                                    stop=sp,
                                )
                    # epilogue for superblock s: rhs = agg + y_own*(w_self*dis^2)
                    # (edge-side dis norms are folded into the one-hot weights)
                    cols = slice(s * SB_D, s * SB_D + ncols)
                    u = epip.tile([PB, ncols], bf16, tag="u")
                    nc.vector.tensor_tensor(
                        out=u[:], in0=ynx_all[:, cols], in1=sdis_t[:, cols],
                        op=mybir.AluOpType.mult,
                    )
                    rhs = epip.tile([PB, ncols], f32, tag="rhs")
                    nc.vector.tensor_tensor(
                        out=rhs[:], in0=ps[:], in1=u[:],
                        op=mybir.AluOpType.add,
                    )
                    zps = psB.tile([outp, ncols], f32, tag="z")
                    for half in range((ncols + 511) // 512):
                        hc = slice(half * 512, min(ncols, (half + 1) * 512))
                        nc.tensor.matmul(
                            zps[:, hc], w_t[layer][:], rhs[:, hc],
                            start=True, stop=True,
                        )
                    if layer < 2:
                        nc.scalar.activation(
                            ynx_all[:, cols], zps[:],
                            mybir.ActivationFunctionType.Relu,
                            bias=b_t[layer][:], scale=1.0,
                        )
                        ynode = epip.tile([PB, ncols // PB, PB], bf16, tag="ynode")
                        for i in range(ncols // PB):
                            nc.sync.dma_start(
                                ynode[:, i, :],
                                ynx_all[:, s * SB_D + i * PB : s * SB_D + (i + 1) * PB],
                                transpose="notranspose" not in ABLATE,
                            )
                        dview = ys[layer][
                            s * SB_D : s * SB_D + ncols, :
                        ].rearrange("(i p) f -> p i f", p=PB)
                        nc.sync.dma_start(dview, ynode[:])
                    else:
                        ot = epip.tile([DOUT, ncols], f32, tag="ot")
                        nc.vector.tensor_scalar(
                            ot[:], zps[:], b_t[layer][:], None,
                            op0=mybir.AluOpType.add,
                        )
                        nc.sync.dma_start(out_t[:, cols], ot[:])
                if layer < 2:
                    if "nocollective" in ABLATE:
                        nc.sync.dma_start(yf[layer][:SHARD_PAD, :], ys[layer][:])
                    else:
                        nc.gpsimd.collective_compute(
                            "AllGather",
                            mybir.AluOpType.bypass,
                            ins=[ys[layer][:]],
                            outs=[yf[layer][:]],
                            replica_groups=[list(range(CORES))],
                        )
    nc.compile()
    return nc


def prepare(x, edge_index, edge_weight, W0, b0, W1, b1, W2, b2, cfg):
    N, D, CORES, SHARD = cfg["N"], cfg["D"], cfg["CORES"], cfg["SHARD"]
    SHARD_PAD, NPAD = cfg["SHARD_PAD"], cfg["NPAD"]
    src = np.asarray(edge_index[0], np.int64)
    dst = np.asarray(edge_index[1], np.int64)
    ew = np.asarray(edge_weight, np.float32)
    x = np.asarray(x, np.float32)

    deg = np.bincount(dst, weights=ew.astype(np.float64), minlength=N)
    dis = np.where(deg > 0, 1.0 / np.sqrt(deg), 0.0).astype(np.float32)

    # normalization folded into the edge weights: table holds raw h
    ytab = np.zeros((NPAD, D), ml_dtypes.bfloat16)
    pr = pad_row(np.arange(N), cfg)
    ytab[pr] = x.astype(ml_dtypes.bfloat16)

    # self edges (src==dst) are applied analytically in the epilogue
    self_m = src == dst
    wself = np.bincount(
        dst[self_m], weights=ew[self_m].astype(np.float64), minlength=N
    ).astype(np.float32)
    nsrc, ndst, new = src[~self_m], dst[~self_m], ew[~self_m]
    new = new * dis[ndst] * dis[nsrc]
    use_w = True

    runs, chunks, idxr, dstv, wval = make_schedule(nsrc, ndst, new, cfg)

    iota = np.tile(np.arange(2 * PB, dtype=np.float32), (PB, 1)).astype(
        ml_dtypes.bfloat16
    )
    sdisb = np.zeros((CORES, PB, SHARD_PAD), ml_dtypes.bfloat16)
    for c in range(CORES):
        sh = slice(c * SHARD, (c + 1) * SHARD)
        sdisb[c, :, :SHARD] = (wself[sh] * dis[sh] * dis[sh]).astype(
            ml_dtypes.bfloat16
        )[None, :]

    shared = {
        "ytab0": ytab,
        "iota": iota,
        "W0": np.asarray(W0, np.float32),
        "W1": np.asarray(W1, np.float32),
        "W2": np.asarray(W2, np.float32),
        "b0": np.asarray(b0, np.float32).reshape(-1, 1),
        "b1": np.asarray(b1, np.float32).reshape(-1, 1),
        "b2": np.asarray(b2, np.float32).reshape(-1, 1),
    }
    in_maps = []
    for c in range(CORES):
        m = dict(shared)
        m["ytab_own"] = ytab[c * SHARD_PAD : (c + 1) * SHARD_PAD]
        m["idxr"] = idxr[c]
        m["dstvr"] = dstv[c]
        if use_w:
            m["wvalr"] = wval[c]
        m["selfdisb"] = sdisb[c]
        in_maps.append(m)
    return runs, chunks, in_maps, idxr.shape[2], use_w


def assemble(results, cfg):
    N, DOUT, CORES, SHARD = cfg["N"], cfg["DOUT"], cfg["CORES"], cfg["SHARD"]
    out = np.empty((N, DOUT), np.float32)
    for c in range(CORES):
        out[c * SHARD : (c + 1) * SHARD] = results[c]["out_t"][:, :SHARD].T
    return out


def run(inputs, cfg=None, trace=False, sim=False):
    cfg = cfg or make_cfg()
    runs, chunks, in_maps, idxcols, use_w = prepare(cfg=cfg, **inputs)
    nc = build_program(runs, chunks, cfg, idxcols, use_w)
    if sim:
        from concourse.bass_interp import MultiCoreSim

        msim = MultiCoreSim(nc, cfg["CORES"])
        for c in range(cfg["CORES"]):
            for k, v in in_maps[c].items():
                msim.cores[c].tensor(k)[:] = v
        msim.simulate()
        results = [
            {"out_t": np.asarray(msim.cores[c].tensor("out_t"))}
            for c in range(cfg["CORES"])
        ]
        return assemble(results, cfg), msim
    try:
        res = run_bass_kernel_spmd(
            nc, in_maps, list(range(cfg["CORES"])), trace=trace
        )
    except ModuleNotFoundError:
        # NTFF profiling hook unavailable in this container
        res = run_bass_kernel_spmd(nc, in_maps, list(range(cfg["CORES"])))
    return assemble(res.results, cfg), res


def kernel(**inputs):
    out, _ = run(inputs)
    return out



# revision 12
# speedup vs baseline: 1.0912x; 1.0113x over previous
"""3-layer GCN on 8 TRN2 NeuronCores (Bass/Tile).

Strategy (matches the sharding hint): nodes are partitioned across the 8
cores (12500 dst nodes each); each core owns the edges whose dst falls in
its shard. Per layer the core aggregates messages for its dst shard by
gathering source-node feature rows (dma_gather, bf16) and reducing them
into PSUM via one-hot matmuls on the TensorEngine; the dense transforms
(agg @ W + b, relu) run on the shard. Node features for the next layer
are exchanged with an AllGather. The full GCN normalization
dis[dst]*w*dis[src] (dis = deg^-1/2) is folded into the per-edge one-hot
values host-side (is_equal * wval), so the table holds raw h and the
epilogue needs only the self-loop term rhs = agg + h_own*(w_self*dis^2);
this is exact because the dense transform commutes with aggregation.

The per-(superblock, src-quadrant) chunk schedule is computed jointly
across all 8 cores so a single SPMD program fits every core; cores pad
their chunks (idx=0 rows with zero one-hot weight) where their edge
counts differ.
"""

import os
import sys

sys.path.insert(0, "/opt/trn_rl_repo")

ABLATE = set(os.environ.get("GCN_ABLATE", "").split(","))
_BUFS = {}
for kv in os.environ.get("GCN_BUFS", "").split(","):
    if "=" in kv:
        k, v = kv.split("=")
        _BUFS[k] = int(v)

import numpy as np
import ml_dtypes

import concourse.bacc as bacc
import concourse.mybir as mybir
import concourse.tile as tile
from concourse import library_config
from concourse.bass_utils import run_bass_kernel_spmd

f32 = mybir.dt.float32
bf16 = mybir.dt.bfloat16
i16 = mybir.dt.int16

PB = 128  # partition / block size


def make_cfg(N=100000, D=128, DOUT=64, CORES=8, SB_BLOCKS=8, NQ=4):
    shard = N // CORES
    assert shard * CORES == N
    shard_pad = ((shard + PB - 1) // PB) * PB
    npad = shard_pad * CORES
    assert npad % NQ == 0
    qrows = npad // NQ
    assert qrows <= 32768, "dma_gather int16 index range"
    nblocks = shard_pad // PB
    sb_d = SB_BLOCKS * PB
    nsb = (nblocks + SB_BLOCKS - 1) // SB_BLOCKS
    return dict(
        N=N, D=D, DOUT=DOUT, CORES=CORES, SHARD=shard, SHARD_PAD=shard_pad,
        NPAD=npad, NQ=NQ, QROWS=qrows, NBLOCKS=nblocks, SB_D=sb_d, NSB=nsb,
    )


def pad_row(n, cfg):
    return (n // cfg["SHARD"]) * cfg["SHARD_PAD"] + n % cfg["SHARD"]


def make_schedule(edge_src, edge_dst, edge_w, cfg):
    """Cross-core-uniform chunk schedule.

    Returns (runs, chunks, per-core arrays):
      runs: list of (sb, q, C, idx_off) in emission order
      chunks: list of (sb, q, wbase, flags) where flags = ((start0, stop0),
              (start1, stop1)) for the two 128-wide matmul halves
      idxr [CORES, 128, idxcols] i16, dstv/wval [CORES, 128, Ctot] f32
    """
    CORES, SHARD, SB_D = cfg["CORES"], cfg["SHARD"], cfg["SB_D"]
    QROWS, NSB, NBLOCKS = cfg["QROWS"], cfg["NSB"], cfg["NBLOCKS"]
    NQ = cfg["NQ"]

    per = []  # [c][sb][q] -> (dsl, idxq, wv) sorted by dsl
    for c in range(CORES):
        m = (edge_dst >= c * SHARD) & (edge_dst < (c + 1) * SHARD)
        dl = edge_dst[m] - c * SHARD
        sp = pad_row(edge_src[m], cfg)
        wv = edge_w[m]
        sb = dl // SB_D
        q = sp // QROWS
        order = np.lexsort((dl, q, sb))
        dl, sp, wv, sb, q = dl[order], sp[order], wv[order], sb[order], q[order]
        key = sb * NQ + q
        bounds = np.searchsorted(key, np.arange(NSB * NQ + 1))
        rows = []
        for s in range(NSB):
            qs = []
            for qq in range(NQ):
                lo, hi = bounds[s * NQ + qq], bounds[s * NQ + qq + 1]
                qs.append((
                    (dl[lo:hi] - s * SB_D).astype(np.int32),
                    (sp[lo:hi] - qq * QROWS).astype(np.int32),
                    wv[lo:hi].astype(np.float32),
                ))
            rows.append(qs)
        per.append(rows)

    runs = []
    chunks = []
    ch_dstv, ch_wval = [], []   # per chunk: [CORES,128] arrays
    run_idx_blocks = []         # per run: [CORES, 128, 8*C] i16
    first_mm = {}               # (sb, bank) -> chunk half getting start
    last_mm = {}
    idx_off = 0
    for s in range(NSB):
        ncols = min(SB_D, NBLOCKS * PB - s * SB_D)
        for qq in range(NQ):
            datas = [per[c][s][qq] for c in range(CORES)]
            lens = [d[0].shape[0] for d in datas]
            if max(lens) == 0:
                continue
            ptr = [0] * CORES
            run_chunk_idx = []  # [CORES,128] per chunk
            C = 0
            while True:
                active = [c for c in range(CORES) if ptr[c] < lens[c]]
                if not active:
                    break
                mind = min(int(datas[c][0][ptr[c]]) for c in active)
                wbase = min((mind // PB) * PB, max(0, ncols - 2 * PB))
                dv = np.full((CORES, PB), -1.0, np.float32)
                wv = np.zeros((CORES, PB), np.float32)
                ix = np.zeros((CORES, PB), np.int16)
                for c in range(CORES):
                    dl, iq, ww = datas[c]
                    p = ptr[c]
                    hi = np.searchsorted(dl, wbase + 2 * PB, side="left")
                    take = min(PB, hi - p)
                    if take > 0:
                        dv[c, :take] = dl[p : p + take] - wbase
                        wv[c, :take] = ww[p : p + take]
                        ix[c, :take] = iq[p : p + take]
                        ptr[c] = p + take
                gc = len(chunks)
                used = [
                    bool(((dv >= 0) & (dv < PB)).any()),
                    bool((dv >= PB).any()),
                ]
                flags = []
                for h in range(2):
                    col = wbase + h * PB
                    if col >= ncols or not used[h]:
                        flags.append(None)
                        continue
                    bank = (s, col // 512)
                    st = bank not in first_mm
                    if st:
                        first_mm[bank] = (gc, h)
                    last_mm[bank] = (gc, h)
                    flags.append(st)
                chunks.append([s, qq, wbase, flags])
                ch_dstv.append(dv)
                ch_wval.append(wv)
                run_chunk_idx.append(ix)
                C += 1
                if C > (max(lens) // PB) + NBLOCKS + 16:
                    raise RuntimeError("chunk packing did not converge")
            # idx region for the run: flat [128*C] -> [16, 8C] -> [128, 8C]
            blk = np.zeros((CORES, PB, 8 * C), np.int16)
            for c in range(CORES):
                flat = np.concatenate([ci[c] for ci in run_chunk_idx])
                wrapped = flat.reshape(-1, 16).T  # [16, 8C]
                blk[c] = np.tile(wrapped, (8, 1))
            run_idx_blocks.append(blk)
            runs.append([s, qq, C, idx_off])
            idx_off += 8 * C

    # every (sb, bank) must receive at least one matmul (else stale PSUM)
    for s in range(NSB):
        ncols = min(SB_D, NBLOCKS * PB - s * SB_D)
        for bank in range((ncols + 511) // 512):
            assert (s, bank) in first_mm, f"uncovered psum bank {(s, bank)}"

    # stop flags
    stops = {v: k for k, v in last_mm.items()}
    for gc, ch in enumerate(chunks):
        fl = ch[3]
        ch[3] = tuple(
            None if fl[h] is None else (fl[h], (gc, h) in stops) for h in range(2)
        )

    Ctot = len(chunks)
    dstv = np.stack(ch_dstv, axis=2)  # [CORES, 128, Ctot]
    wval = np.stack(ch_wval, axis=2)
    idxr = np.concatenate(run_idx_blocks, axis=2)  # [CORES, 128, idxcols]
    return runs, chunks, idxr, dstv, wval


def build_program(runs, chunks, cfg, idxcols, use_w):
    CORES, D, DOUT = cfg["CORES"], cfg["D"], cfg["DOUT"]
    SHARD_PAD, NPAD, QROWS = cfg["SHARD_PAD"], cfg["NPAD"], cfg["QROWS"]
    NSB, SB_D, NBLOCKS, NQ = cfg["NSB"], cfg["SB_D"], cfg["NBLOCKS"], cfg["NQ"]
    Ctot = len(chunks)
    Cmax = max(r[2] for r in runs)

    nc = bacc.Bacc("TRN2", debug=False)
    ytab0 = nc.dram_tensor("ytab0", [NPAD, D], bf16, kind="ExternalInput")
    ytab_own = nc.dram_tensor("ytab_own", [SHARD_PAD, D], bf16, kind="ExternalInput")
    idx_in = nc.dram_tensor("idxr", [PB, idxcols], i16, kind="ExternalInput")
    dstv_in = nc.dram_tensor("dstvr", [PB, Ctot], f32, kind="ExternalInput")
    if use_w:
        wval_in = nc.dram_tensor("wvalr", [PB, Ctot], f32, kind="ExternalInput")
    sdis_in = nc.dram_tensor("selfdisb", [PB, SHARD_PAD], bf16, kind="ExternalInput")
    iota_in = nc.dram_tensor("iota", [PB, 2 * PB], bf16, kind="ExternalInput")
    w_in = [
        nc.dram_tensor("W0", [D, D], f32, kind="ExternalInput"),
        nc.dram_tensor("W1", [D, D], f32, kind="ExternalInput"),
        nc.dram_tensor("W2", [D, DOUT], f32, kind="ExternalInput"),
    ]
    b_in = [
        nc.dram_tensor("b0", [D, 1], f32, kind="ExternalInput"),
        nc.dram_tensor("b1", [D, 1], f32, kind="ExternalInput"),
        nc.dram_tensor("b2", [DOUT, 1], f32, kind="ExternalInput"),
    ]
    out_t = nc.dram_tensor("out_t", [DOUT, SHARD_PAD], f32, kind="ExternalOutput")
    ys = [nc.dram_tensor(f"ys{l}", [SHARD_PAD, D], bf16) for l in range(2)]
    yf = [
        nc.dram_tensor(f"yf{l}", [NPAD, D], bf16, addr_space="Shared")
        for l in range(2)
    ]

    with tile.TileContext(nc) as tc:
        with (
            tc.tile_pool(name="const", bufs=1) as constp,
            tc.tile_pool(name="gat", bufs=_BUFS.get("gat", 6)) as gatp,
            tc.tile_pool(name="ohp", bufs=_BUFS.get("oh", 14)) as ohp,
            tc.tile_pool(name="epi", bufs=_BUFS.get("epi", 3)) as epip,
            tc.tile_pool(name="psA", bufs=_BUFS.get("psA", 2), space="PSUM") as psA,
            tc.tile_pool(name="psB", bufs=1, space="PSUM") as psB,
        ):
            nc.gpsimd.load_library(library_config.mlp)
            idx_t = constp.tile([PB, idxcols], i16)
            dstv_t = constp.tile([PB, Ctot], f32)
            sdis_t = constp.tile([PB, SHARD_PAD], bf16)
            iota_t = constp.tile([PB, 2 * PB], bf16)
            ynx_all = constp.tile([PB, SHARD_PAD], bf16)
            nc.sync.dma_start(idx_t[:], idx_in[:])
            nc.sync.dma_start(dstv_t[:], dstv_in[:])
            if use_w:
                wval_t = constp.tile([PB, Ctot], f32)
                nc.sync.dma_start(wval_t[:], wval_in[:])
            nc.sync.dma_start(sdis_t[:], sdis_in[:])
            nc.sync.dma_start(iota_t[:], iota_in[:])
            for i in range(SHARD_PAD // PB):
                nc.sync.dma_start(
                    ynx_all[:, i * PB : (i + 1) * PB],
                    ytab_own[i * PB : (i + 1) * PB, :],
                    transpose="notranspose" not in ABLATE,
                )
            w_t = []
            b_t = []
            for l in range(3):
                wt = constp.tile(list(w_in[l].shape), f32)
                bt = constp.tile(list(b_in[l].shape), f32)
                nc.sync.dma_start(wt[:], w_in[l][:])
                nc.sync.dma_start(bt[:], b_in[l][:])
                w_t.append(wt)
                b_t.append(bt)

            # group runs/chunks by sb
            run_by_sb = [[] for _ in range(NSB)]
            for ri, r in enumerate(runs):
                run_by_sb[r[0]].append(ri)
            chunk_of_run = [[] for _ in runs]
            for gc, ch in enumerate(chunks):
                # chunks are appended run-major in schedule order
                pass
            # recompute chunk->run mapping from order
            gc = 0
            for ri, r in enumerate(runs):
                for j in range(r[2]):
                    chunk_of_run[ri].append(gc)
                    gc += 1

            for layer in range(3):
                table = [ytab0, yf[0], yf[1]][layer]
                relu = layer < 2
                outp = D if layer < 2 else DOUT
                for s in range(NSB):
                    ncols = min(SB_D, NBLOCKS * PB - s * SB_D)
                    ps = psA.tile([PB, ncols], f32, tag="agg")
                    for ri in run_by_sb[s]:
                        _, qq, C, ioff = runs[ri]
                        gt = gatp.tile([PB, C, D], bf16, tag="gt")
                        nc.gpsimd.dma_gather(
                            gt[:],
                            table[qq * QROWS : (qq + 1) * QROWS, :],
                            idx_t[:, ioff : ioff + 8 * C],
                            PB * C,
                            PB * C,
                            D,
                            single_packet="singlepacket" in ABLATE,
                        )
                        for j, gc in enumerate(chunk_of_run[ri]):
                            _, _, wbase, flags = chunks[gc]
                            if flags[0] is None and flags[1] is None:
                                continue  # pure-pad chunk
                            lo = 0 if flags[0] is not None else PB
                            hi = 2 * PB if flags[1] is not None else PB
                            oh = ohp.tile([PB, 2 * PB], bf16, tag="oh")
                            if use_w:
                                nc.vector.tensor_scalar(
                                    oh[:, lo:hi],
                                    iota_t[:, lo:hi],
                                    dstv_t[:, gc : gc + 1],
                                    wval_t[:, gc : gc + 1],
                                    op0=mybir.AluOpType.is_equal,
                                    op1=mybir.AluOpType.mult,
                                )
                            else:
                                nc.vector.tensor_scalar(
                                    oh[:, lo:hi],
                                    iota_t[:, lo:hi],
                                    dstv_t[:, gc : gc + 1],
                                    None,
                                    op0=mybir.AluOpType.is_equal,
                                )
                            for h in range(2):
                                if flags[h] is None:
                                    continue
                                st, sp = flags[h]
                                nc.tensor.matmul(
                                    ps[:, wbase + h * PB : wbase + (h + 1) * PB],
                                    gt[:, j, :],
                                    oh[:, h * PB : (h + 1) * PB],
                                    start=st,
                                    stop=sp,
                                )
                    # epilogue for superblock s: rhs = agg + y_own*(w_self*dis^2)
                    # (edge-side dis norms are folded into the one-hot weights)
                    cols = slice(s * SB_D, s * SB_D + ncols)
                    u = epip.tile([PB, ncols], bf16, tag="u")
                    nc.vector.tensor_tensor(
                        out=u[:], in0=ynx_all[:, cols], in1=sdis_t[:, cols],
                        op=mybir.AluOpType.mult,
                    )
                    rhs = epip.tile([PB, ncols], f32, tag="rhs")
                    nc.vector.tensor_tensor(
                        out=rhs[:], in0=ps[:], in1=u[:],
                        op=mybir.AluOpType.add,
                    )
                    zps = psB.tile([outp, ncols], f32, tag="z")
                    for half in range((ncols + 511) // 512):
                        hc = slice(half * 512, min(ncols, (half + 1) * 512))
                        nc.tensor.matmul(
                            zps[:, hc], w_t[layer][:], rhs[:, hc],
                            start=True, stop=True,
                        )
                    if layer < 2:
                        nc.scalar.activation(
                            ynx_all[:, cols], zps[:],
                            mybir.ActivationFunctionType.Relu,
                            bias=b_t[layer][:], scale=1.0,
                        )
                        ynode = epip.tile([PB, ncols // PB, PB], bf16, tag="ynode")
                        for i in range(ncols // PB):
                            nc.sync.dma_start(
                                ynode[:, i, :],
                                ynx_all[:, s * SB_D + i * PB : s * SB_D + (i + 1) * PB],
                                transpose="notranspose" not in ABLATE,
                            )
                        dview = ys[layer][
                            s * SB_D : s * SB_D + ncols, :
                        ].rearrange("(i p) f -> p i f", p=PB)
                        nc.sync.dma_start(dview, ynode[:])
                    else:
                        ot = epip.tile([DOUT, ncols], f32, tag="ot")
                        nc.scalar.activation(
                            ot[:], zps[:],
                            mybir.ActivationFunctionType.Identity,
                            bias=b_t[layer][:], scale=1.0,
                        )
                        nc.sync.dma_start(out_t[:, cols], ot[:])
                if layer < 2:
                    if "nocollective" in ABLATE:
                        nc.sync.dma_start(yf[layer][:SHARD_PAD, :], ys[layer][:])
                    else:
                        nc.gpsimd.collective_compute(
                            "AllGather",
                            mybir.AluOpType.bypass,
                            ins=[ys[layer][:]],
                            outs=[yf[layer][:]],
                            replica_groups=[list(range(CORES))],
                        )
    nc.compile()
    return nc


def prepare(x, edge_index, edge_weight, W0, b0, W1, b1, W2, b2, cfg):
    N, D, CORES, SHARD = cfg["N"], cfg["D"], cfg["CORES"], cfg["SHARD"]
    SHARD_PAD, NPAD = cfg["SHARD_PAD"], cfg["NPAD"]
    src = np.asarray(edge_index[0], np.int64)
    dst = np.asarray(edge_index[1], np.int64)
    ew = np.asarray(edge_weight, np.float32)
    x = np.asarray(x, np.float32)

    deg = np.bincount(dst, weights=ew.astype(np.float64), minlength=N)
    dis = np.where(deg > 0, 1.0 / np.sqrt(deg), 0.0).astype(np.float32)

    # normalization folded into the edge weights: table holds raw h
    ytab = np.zeros((NPAD, D), ml_dtypes.bfloat16)
    pr = pad_row(np.arange(N), cfg)
    ytab[pr] = x.astype(ml_dtypes.bfloat16)

    # self edges (src==dst) are applied analytically in the epilogue
    self_m = src == dst
    wself = np.bincount(
        dst[self_m], weights=ew[self_m].astype(np.float64), minlength=N
    ).astype(np.float32)
    nsrc, ndst, new = src[~self_m], dst[~self_m], ew[~self_m]
    new = new * dis[ndst] * dis[nsrc]
    use_w = True

    runs, chunks, idxr, dstv, wval = make_schedule(nsrc, ndst, new, cfg)

    iota = np.tile(np.arange(2 * PB, dtype=np.float32), (PB, 1)).astype(
        ml_dtypes.bfloat16
    )
    sdisb = np.zeros((CORES, PB, SHARD_PAD), ml_dtypes.bfloat16)
    for c in range(CORES):
        sh = slice(c * SHARD, (c + 1) * SHARD)
        sdisb[c, :, :SHARD] = (wself[sh] * dis[sh] * dis[sh]).astype(
            ml_dtypes.bfloat16
        )[None, :]

    shared = {
        "ytab0": ytab,
        "iota": iota,
        "W0": np.asarray(W0, np.float32),
        "W1": np.asarray(W1, np.float32),
        "W2": np.asarray(W2, np.float32),
        "b0": np.asarray(b0, np.float32).reshape(-1, 1),
        "b1": np.asarray(b1, np.float32).reshape(-1, 1),
        "b2": np.asarray(b2, np.float32).reshape(-1, 1),
    }
    in_maps = []
    for c in range(CORES):
        m = dict(shared)
        m["ytab_own"] = ytab[c * SHARD_PAD : (c + 1) * SHARD_PAD]
        m["idxr"] = idxr[c]
        m["dstvr"] = dstv[c]
        if use_w:
            m["wvalr"] = wval[c]
        m["selfdisb"] = sdisb[c]
        in_maps.append(m)
    return runs, chunks, in_maps, idxr.shape[2], use_w


def assemble(results, cfg):
    N, DOUT, CORES, SHARD = cfg["N"], cfg["DOUT"], cfg["CORES"], cfg["SHARD"]
    out = np.empty((N, DOUT), np.float32)
    for c in range(CORES):
        out[c * SHARD : (c + 1) * SHARD] = results[c]["out_t"][:, :SHARD].T
    return out


def run(inputs, cfg=None, trace=False, sim=False):
    cfg = cfg or make_cfg()
    runs, chunks, in_maps, idxcols, use_w = prepare(cfg=cfg, **inputs)
    nc = build_program(runs, chunks, cfg, idxcols, use_w)
    if sim:
        from concourse.bass_interp import MultiCoreSim

        msim = MultiCoreSim(nc, cfg["CORES"])
        for c in range(cfg["CORES"]):
            for k, v in in_maps[c].items():
                msim.cores[c].tensor(k)[:] = v
        msim.simulate()
        results = [
            {"out_t": np.asarray(msim.cores[c].tensor("out_t"))}
            for c in range(cfg["CORES"])
        ]
        return assemble(results, cfg), msim
    try:
        res = run_bass_kernel_spmd(
            nc, in_maps, list(range(cfg["CORES"])), trace=trace
        )
    except ModuleNotFoundError:
        # NTFF profiling hook unavailable in this container
        res = run_bass_kernel_spmd(nc, in_maps, list(range(cfg["CORES"])))
    return assemble(res.results, cfg), res


def kernel(**inputs):
    out, _ = run(inputs)
    return out

